# revision 20
# baseline (speedup 1.0000x reference)
"""Distributed 2-layer GAT on 8 Trainium2 NeuronCores (Bass/Tile), v7.

Strategy (graph/data parallel, dst-aligned edge grids):
  - Nodes sharded across 8 cores (6250 each, padded to 6272 = 49*128).
  - Per core, nodes are packed into 49 tiles of 128 by iterated (lo,hi)
    gather-load sorting, so each tile's max per-window in-degree (the
    edge-grid width K_t) stays tight; K unified across cores (SPMD).
  - Edge grids are DST-ALIGNED: partition i holds only edges whose
    destination is the tile's node at row i.  al_dst is a per-partition
    broadcast add, the softmax denominator a row reduce, the weighted
    scatter-sum K accumulating PE matmuls with a constant bf16 identity.
  - Node tables ([h | a_src] rows, bf16) are AllGathered in 7 chunks that
    pipeline with the producing compute, restrided into contiguous
    ExternalOutput buffers (the dma_gather ucode needs jax-allocated
    contiguous tables), then fetched per edge with one dma_gather per
    (tile, index-window).  int16 gather indices -> two overlapping 32768-
    row windows; edges from the 15360-row overlap balance the two calls.
  - Pad slots point at rows whose a_src columns hold -1e9 (exact 0 weight).
"""

import os
import sys
import types

import numpy as np

_BUILD_CACHE = {}

C = 8
N, F, HEADS, HID, NCLS = 50000, 512, 4, 64, 64
D1 = HEADS * HID        # 256
NPC = N // C            # 6250
NT = 49
PADN = NT * 128         # 6272
NPALL = C * PADN        # 50176
NCH = 7                 # allgather chunks (7 tiles each)
TPC = NT // NCH         # tiles per chunk
CROWS = TPC * 128       # 896 rows per (core, chunk)
CHB = C * CROWS         # 7168 rows per chunk block in the global table
HALF = 32768            # int16 index limit for dma_gather
HIBASE = NPALL - HALF   # hi-window base (17408); windows overlap 15360 rows
S1 = 384                # layer-1 gather-table row stride (bf16 cols; 768B)
G1 = D1 + HEADS         # gathered layer-1 row width: 260
S2 = 128                # layer-2 gather-table row stride (bf16 cols; 256B)
G2 = NCLS + 1           # gathered layer-2 row width: 65
TW1 = D1 + 2 * HEADS    # phase-A matmul width 264
TW2 = NCLS + 2          # phase-D matmul width 66
FK = F // 128           # 4
DK = D1 // 128          # 2
NEG = -1e9
PADROW0 = 127           # tile-0 pad row (every core; rank gap at 127)
PADROWL = (NPC + 1) % 128  # 107: first pad row within the last tile


def _grow(core, posn):
    """Global gather-table row for (core, in-core position): chunk-major."""
    t = posn // 128
    return (t // TPC) * CHB + core * CROWS + (t % TPC) * 128 + posn % 128


def _register_trace_hook():
    try:
        if "antenv.axon_hooks" in sys.modules:
            return True
        from trn_agent_boot.trn_boot import _ntff_profile_via_ctypes

        hook = _ntff_profile_via_ctypes("/opt/axon/libaxon_pjrt.so")
        m = types.ModuleType("antenv.axon_hooks")
        m.get_axon_ntff_profile_hook = lambda: hook
        m.set_axon_ntff_profile_hook = lambda h: None
        sys.modules["antenv.axon_hooks"] = m
        return True
    except Exception:
        return False


def _wrap16(flat):
    """dma_gather index layout: wrapped in 16 partitions, replicated x8."""
    n = len(flat)
    assert n % 16 == 0
    w = np.asarray(flat, np.int64).reshape(n // 16, 16).T.astype(np.int32)
    w = np.tile(w, (8, 1))
    return w.astype(np.uint16).view(np.int16)


def _host_prep(x, edge_index, W1, a_src1, a_dst1, b1, W2, a_src2, a_dst2, b2):
    import ml_dtypes
    bf = ml_dtypes.bfloat16

    x = np.asarray(x, np.float32)
    ei = np.asarray(edge_index)
    W1 = np.asarray(W1, np.float32)
    a_src1 = np.asarray(a_src1, np.float32)
    a_dst1 = np.asarray(a_dst1, np.float32)
    b1 = np.asarray(b1, np.float32)
    W2 = np.asarray(W2, np.float32)
    a_src2 = np.asarray(a_src2, np.float32)
    a_dst2 = np.asarray(a_dst2, np.float32)
    b2 = np.asarray(b2, np.float32)

    src = np.concatenate([ei[0], np.arange(N)]).astype(np.int64)
    dst = np.concatenate([ei[1], np.arange(N)]).astype(np.int64)
    deg = np.bincount(dst, minlength=N).astype(np.int64)
    ncidx = np.arange(N) // NPC

    def assign_rows(keys):
        """rank -> position, skipping position 127 (the lo-window pad row)."""
        p = np.empty(N, np.int64)
        for c in range(C):
            s = slice(c * NPC, (c + 1) * NPC)
            order = np.lexsort(tuple(k[s] for k in keys))
            q = np.empty(NPC, np.int64)
            r = np.arange(NPC)
            q[order] = r + (r >= PADROW0)
            p[s] = q
        return p

    def window_loads(pos_):
        grow_ = _grow(ncidx, pos_)
        gs_ = grow_[src]
        gd_ = grow_[dst]
        zone_ = (gs_ >= HIBASE).astype(np.int64) + (gs_ >= HALF).astype(np.int64)
        cnt = np.bincount(gd_ * 3 + zone_, minlength=NPALL * 3).reshape(NPALL, 3)
        n1, nf, n2 = cnt[:, 0], cnt[:, 1], cnt[:, 2]
        xf = np.clip((n2 + nf - n1 + 1) // 2, 0, nf)
        return n1 + xf, n2 + nf - xf

    pos = assign_rows((-deg,))
    for _ in range(4):
        wlo_t, whi_t = window_loads(pos)
        g_ = _grow(ncidx, pos)
        pos = assign_rows((-whi_t[g_], -wlo_t[g_]))

    grow = _grow(ncidx, pos)
    node_at = np.full((C, PADN), -1, np.int64)
    node_at[ncidx, pos] = np.arange(N)

    # --- per-(core,tile,row) edge grouping into overlapping lo/hi windows ---
    gs = grow[src]
    lrow = ncidx[dst] * PADN + pos[dst]              # (core, local row)
    zone = (gs >= HIBASE).astype(np.int64) + (gs >= HALF).astype(np.int64)
    okey = np.lexsort((gs, zone, lrow))
    gs_s = gs[okey]
    key_s = (lrow * 3 + zone)[okey]
    bounds = np.searchsorted(key_s, np.arange(NPALL * 3 + 1))
    d3 = (bounds[1:] - bounds[:-1]).reshape(NPALL, 3)
    n1, nf, n2 = d3[:, 0], d3[:, 1], d3[:, 2]
    xfl = np.clip((n2 + nf - n1 + 1) // 2, 0, nf)
    wlo = n1 + xfl
    whi = n2 + nf - xfl
    KLO = np.maximum(wlo.reshape(C, NT, 128).max(axis=2).max(axis=0), 1)
    KHI = np.maximum(whi.reshape(C, NT, 128).max(axis=2).max(axis=0), 1)

    PADLO = PADROW0                          # core-0 tile-0 row-127 pad
    PADHI = _grow(7, NPC + 1) - HIBASE       # core-7 last-tile pad, hi-local
    assert 0 <= PADHI < HALF, PADHI

    idx_lo = np.full((C, NT), None, dtype=object)
    idx_hi = np.full((C, NT), None, dtype=object)
    for c in range(C):
        for t in range(NT):
            klo, khi = KLO[t], KHI[t]
            glo = np.full((128, klo), PADLO, np.int64)
            ghi = np.full((128, khi), PADHI, np.int64)
            for i in range(128):
                r = (c * PADN + t * 128) + i
                b0, b1_, b2_, b3_ = bounds[3 * r:3 * r + 4]
                xl = xfl[r]
                lo_rows = np.concatenate([gs_s[b0:b1_], gs_s[b1_:b1_ + xl]])
                hi_rows = gs_s[b1_ + xl:b3_]
                glo[i, :len(lo_rows)] = lo_rows
                ghi[i, :len(hi_rows)] = hi_rows - HIBASE
                if b3_ == b0:
                    # padded (node-less) row: one REAL gather keeps its
                    # softmax denominator nonzero (no eps on device)
                    glo[i, 0] = 0
            idx_lo[c, t] = glo.T.ravel()   # column-major = placement order
            idx_hi[c, t] = ghi.T.ravel()

    # --- per-core transposed x shards, bf16, per-tile-contiguous blocks ---
    xs = np.zeros((C, PADN, F), np.float32)
    xs[ncidx, pos] = x
    xsTt = np.ascontiguousarray(
        xs.reshape(C, NT, 128, F).transpose(0, 1, 3, 2)).astype(bf)  # [C,NT,F,128]

    # --- extended weights (bf16) ---
    Wa_s1 = np.einsum("fhc,hc->fh", W1.reshape(F, HEADS, HID), a_src1)
    Wa_d1 = np.einsum("fhc,hc->fh", W1.reshape(F, HEADS, HID), a_dst1)
    W1e = np.ascontiguousarray(
        np.concatenate([W1, Wa_s1, Wa_d1], axis=1)).astype(bf)       # [512,264]
    Wa_s2 = W2 @ a_src2[0]
    Wa_d2 = W2 @ a_dst2[0]
    W2e = np.ascontiguousarray(np.concatenate(
        [W2, Wa_s2[:, None], Wa_d2[:, None]], axis=1)).astype(bf)    # [256,66]
    b1r = np.broadcast_to(b1[None, :], (128, D1)).astype(np.float32).copy()
    b2r = np.broadcast_to(b2[None, :], (128, NCLS)).astype(np.float32).copy()
    ident = np.eye(128, dtype=np.float32).astype(bf)
    padm = np.zeros((128, 2 * HEADS), np.float32)
    padm[PADROW0, 0:HEADS] = NEG           # tile-0 pad row mask
    padm[PADROWL:, HEADS:2 * HEADS] = NEG  # last-tile pad rows mask

    in_maps = []
    for c in range(C):
        in_maps.append({
            "xsTt": xsTt[c],
            "w1e": W1e,
            "w2e": W2e,
            "b1r": b1r,
            "b2r": b2r,
            "ident": np.ascontiguousarray(ident),
            "ilo": np.hstack([_wrap16(idx_lo[c, t]) for t in range(NT)]),
            "ihi": np.hstack([_wrap16(idx_hi[c, t]) for t in range(NT)]),
            "padm": padm,
        })
    cfg = dict(KLO=tuple(int(v) for v in KLO), KHI=tuple(int(v) for v in KHI))
    return cfg, in_maps, node_at


def _dma_gather_raw(nc, out_ap, in_ap, idxs_ap, num_idxs, elem_size, elem_step,
                    queue_num=0):
    """nc.gpsimd.dma_gather minus the elem_size%256 over-assert (the ISA only
    requires the ROW STRIDE to be a 256B multiple; verified on hardware)."""
    import concourse.mybir as mybir
    from concourse.bass import exact_div

    g = nc.gpsimd
    stride_bytes = elem_step * mybir.dt.size(in_ap.dtype)
    stride_bytes_256 = exact_div(stride_bytes, 256)
    _in_ap = g.lower_ap_dma(in_ap, for_custom_bir_dma=True)
    _idxs_ap = g.lower_ap(idxs_ap)
    _out_ap = g.lower_ap(out_ap)
    return g.add_instruction(
        mybir.InstDMAGatherAnt(
            name=g.bass.get_next_instruction_name(),
            ins=[*_in_ap, _idxs_ap, g.lower_val_access(g.to_reg(num_idxs))],
            outs=[_out_ap],
            transpose=False,
            num_idxs=num_idxs,
            elem_size=elem_size,
            stride_bytes_256=stride_bytes_256,
            gen_mode=0,
            single_packet=False,
            queue_num=queue_num,
            sbuf_tokens_per_rank=0,
            sbuf_free_dim_per_rank=0,
            sbuf_free_dim_pad_per_rank=0,
            sbuf_byte_offset=0,
        )
    )


def _build_program(KLO, KHI):
    import concourse.bacc as bacc
    import concourse.bass as bass
    import concourse.mybir as mybir
    import concourse.tile as tile

    f32 = mybir.dt.float32
    bf16 = mybir.dt.bfloat16
    i16 = mybir.dt.int16
    AF = mybir.ActivationFunctionType
    ALU = mybir.AluOpType
    AX = mybir.AxisListType

    KLO = list(KLO)
    KHI = list(KHI)
    K = [a + b for a, b in zip(KLO, KHI)]
    LOFF = np.concatenate([[0], np.cumsum(KLO)]).tolist()
    HOFF = np.concatenate([[0], np.cumsum(KHI)]).tolist()

    NSWQ = int(os.environ.get("GAT_NSWQ", "4"))
    nc = bacc.Bacc("TRN2", target_bir_lowering=False, debug=False,
                   num_devices=C, num_swdge_queues=NSWQ)

    xsTt = nc.dram_tensor("xsTt", [NT, F, 128], bf16, kind="ExternalInput")
    w1e = nc.dram_tensor("w1e", [F, TW1], bf16, kind="ExternalInput")
    w2e = nc.dram_tensor("w2e", [D1, TW2], bf16, kind="ExternalInput")
    b1r = nc.dram_tensor("b1r", [128, D1], f32, kind="ExternalInput")
    b2r = nc.dram_tensor("b2r", [128, NCLS], f32, kind="ExternalInput")
    idn = nc.dram_tensor("ident", [128, 128], bf16, kind="ExternalInput")
    padm = nc.dram_tensor("padm", [128, 2 * HEADS], f32, kind="ExternalInput")
    ilo = nc.dram_tensor("ilo", [128, LOFF[NT] * 8], i16, kind="ExternalInput")
    ihi = nc.dram_tensor("ihi", [128, HOFF[NT] * 8], i16, kind="ExternalInput")
    outp = nc.dram_tensor("outp", [PADN, NCLS], f32, kind="ExternalOutput")

    NT_A = 28               # tiles in AG1a (4 chunks); AG1b gets 21 (3 chunks)
    loc1 = nc.dram_tensor("loc1", [PADN, G1], bf16)
    t1a = nc.dram_tensor("tab1ia", [C * NT_A * 128, G1], bf16,
                         addr_space="Shared")
    t1b = nc.dram_tensor("tab1ib", [C * (NT - NT_A) * 128, G1], bf16,
                         addr_space="Shared")
    tab1 = nc.dram_tensor("tabg1", [NPALL, S1], bf16, kind="ExternalOutput")
    loc2 = [nc.dram_tensor(f"loc2_{j}", [CROWS, S2], bf16) for j in range(NCH)]
    t2i = [nc.dram_tensor(f"tab2i_{j}", [CHB, S2], bf16, addr_space="Shared")
           for j in range(NCH)]
    tab2 = nc.dram_tensor("tabg2", [NPALL, S2], bf16, kind="ExternalOutput")

    rg = [list(range(C))]

    with tile.TileContext(nc) as tc:
        with (
            tc.tile_pool(name="const", bufs=1) as const,
            tc.tile_pool(name="wk", bufs=4) as wk,
            tc.tile_pool(name="gv", bufs=4) as gvp,
            tc.tile_pool(name="ps", bufs=2, space="PSUM") as psp,
            tc.tile_pool(name="pst", bufs=2, space="PSUM") as pstp,
        ):
            # ---- constants ----
            w1s = const.tile([128, FK * TW1], bf16, tag="w1")
            nc.sync.dma_start(
                out=w1s[:].rearrange("p (k c) -> p k c", c=TW1),
                in_=w1e[:, :].rearrange("(k p) c -> p k c", p=128),
            )
            w2s = const.tile([128, DK * TW2], bf16, tag="w2")
            nc.sync.dma_start(
                out=w2s[:].rearrange("p (k c) -> p k c", c=TW2),
                in_=w2e[:, :].rearrange("(k p) c -> p k c", p=128),
            )
            b1s = const.tile([128, D1], f32, tag="b1")
            nc.sync.dma_start(out=b1s[:], in_=b1r[:, :])
            b2s = const.tile([128, NCLS], f32, tag="b2")
            nc.sync.dma_start(out=b2s[:], in_=b2r[:, :])
            ids = const.tile([128, 128], bf16, tag="ident")
            nc.sync.dma_start(out=ids[:], in_=idn[:, :])
            padms = const.tile([128, 2 * HEADS], f32, tag="padm")
            nc.sync.dma_start(out=padms[:], in_=padm[:, :])
            ald1s = const.tile([128, NT * HEADS], f32, tag="ald1")
            ald2s = const.tile([128, NT], f32, tag="ald2")
            zc = const.tile([128, 1], f32, tag="zc")
            nc.vector.memset(zc[:], 0.0)
            ilos = const.tile([128, LOFF[NT] * 8], i16, tag="ilos")
            nc.sync.dma_start(out=ilos[:], in_=ilo[:, :])
            ihis = const.tile([128, HOFF[NT] * 8], i16, tag="ihis")
            nc.sync.dma_start(out=ihis[:], in_=ihi[:, :])
            shs = const.tile([128, NT * NCLS], f32, tag="shs")
            ssums = const.tile([128, NT], f32, tag="ssums")
            lgss = const.tile([128, NT], f32, tag="lgss")

            # ---- phase A: local table 1, chunk-pipelined with AG1+copy ----
            with nc.named_scope("l1_local_mm"):
                for t in range(NT):
                    j, tj = t // TPC, t % TPC
                    xa = wk.tile([128, F], bf16, tag="xa")
                    nc.sync.dma_start(
                        out=xa[:].rearrange("p (k n) -> p k n", n=128),
                        in_=xsTt[t].rearrange("(k p) n -> p k n", p=128),
                    )
                    ps_a = psp.tile([128, TW1], f32, tag="mm")
                    for kk in range(FK):
                        nc.tensor.matmul(
                            ps_a[:],
                            lhsT=xa[:, kk * 128:(kk + 1) * 128],
                            rhs=w1s[:, kk * TW1:(kk + 1) * TW1],
                            start=(kk == 0), stop=(kk == FK - 1),
                        )
                    ha = wk.tile([128, G1], bf16, tag="ha")
                    nc.vector.tensor_copy(ha[:], ps_a[:, 0:G1])
                    if t == 0:
                        nc.vector.tensor_add(ha[:, D1:G1], ha[:, D1:G1],
                                             padms[:, 0:HEADS])
                    if t == NT - 1:
                        nc.vector.tensor_add(ha[:, D1:G1], ha[:, D1:G1],
                                             padms[:, HEADS:2 * HEADS])
                    nc.sync.dma_start(
                        out=loc1[t * 128:(t + 1) * 128, :], in_=ha[:])
                    nc.vector.tensor_copy(
                        ald1s[:, t * HEADS:(t + 1) * HEADS],
                        ps_a[:, G1:G1 + HEADS])
                    if t == NT_A - 1:
                        with nc.named_scope("l1_ag"):
                            nc.gpsimd.collective_compute(
                                "AllGather", mybir.AluOpType.bypass,
                                replica_groups=rg,
                                ins=[loc1[0:NT_A * 128, :]], outs=[t1a[:]],
                            )
                        with nc.named_scope("l1_restride"):
                            for c in range(C):
                                for jj in range(NT_A // TPC):
                                    nc.sync.dma_start(
                                        out=tab1[jj * CHB + c * CROWS:
                                                 jj * CHB + (c + 1) * CROWS,
                                                 0:G1],
                                        in_=t1a[(c * NT_A + jj * TPC) * 128:
                                                (c * NT_A + (jj + 1) * TPC) * 128,
                                                :])
                    if t == NT - 1:
                        with nc.named_scope("l1_ag"):
                            nc.gpsimd.collective_compute(
                                "AllGather", mybir.AluOpType.bypass,
                                replica_groups=rg,
                                ins=[loc1[NT_A * 128:PADN, :]], outs=[t1b[:]],
                            )
                        with nc.named_scope("l1_restride"):
                            NT_B = NT - NT_A
                            for c in range(C):
                                for jj in range(NT_B // TPC):
                                    J = NT_A // TPC + jj
                                    nc.sync.dma_start(
                                        out=tab1[J * CHB + c * CROWS:
                                                 J * CHB + (c + 1) * CROWS,
                                                 0:G1],
                                        in_=t1b[(c * NT_B + jj * TPC) * 128:
                                                (c * NT_B + (jj + 1) * TPC) * 128,
                                                :])

            # ---- phase C: layer-1 edge pass (+ fused layer-2 local mm) ----
            with nc.named_scope("l1_edges"):
                for t in range(NT):
                    j, tj = t // TPC, t % TPC
                    klo, khi, kt = KLO[t], KHI[t], K[t]
                    g = gvp.tile([128, kt * G1], bf16, tag="g1")
                    gv = g[:].rearrange("p (k c) -> p k c", c=G1)
                    _dma_gather_raw(nc, gv[:, 0:klo, :], tab1[0:HALF, 0:G1],
                                    ilos[:, LOFF[t] * 8:LOFF[t + 1] * 8],
                                    klo * 128, G1, S1,
                                    queue_num=(2 * t) % NSWQ)
                    _dma_gather_raw(nc, gv[:, klo:kt, :],
                                    tab1[HIBASE:NPALL, 0:G1],
                                    ihis[:, HOFF[t] * 8:HOFF[t + 1] * 8],
                                    khi * 128, G1, S1,
                                    queue_num=(2 * t + 1) % NSWQ)

                    # attention weights
                    e = wk.tile([128, kt * HEADS], f32, tag="e")
                    ev = e[:].rearrange("p (k h) -> p k h", h=HEADS)
                    adb = ald1s[:, t * HEADS:(t + 1) * HEADS].unsqueeze(1) \
                        .to_broadcast([128, kt, HEADS])
                    nc.vector.tensor_tensor(ev, gv[:, :, D1:G1], adb, op=ALU.add)
                    nc.vector.scalar_tensor_tensor(
                        e[:], e[:], 0.2, e[:], op0=ALU.mult, op1=ALU.max)
                    exw = wk.tile([128, kt * HEADS], bf16, tag="exw")
                    nc.scalar.activation(exw[:], e[:], AF.Exp)

                    den = wk.tile([128, HEADS], f32, tag="den")
                    nc.vector.tensor_reduce(
                        den[:], exw[:].rearrange("p (k h) -> p h k", h=HEADS),
                        axis=AX.X, op=ALU.add)
                    rec = wk.tile([128, HEADS], f32, tag="rec")
                    nc.vector.reciprocal(rec[:], den[:])

                    # weight rows in place, then identity-scatter on PE
                    gf = gv[:, :, 0:D1].rearrange("p k (h c) -> p k h c", c=HID)
                    exb = exw[:].rearrange("p (k h) -> p k h", h=HEADS) \
                        .unsqueeze(3).to_broadcast([128, kt, HEADS, HID])
                    nc.vector.tensor_mul(gf, gf, exb)

                    ps_c = psp.tile([128, D1], f32, tag="mm")
                    for k in range(kt):
                        nc.tensor.matmul(
                            ps_c[:], lhsT=ids[:], rhs=gv[:, k, 0:D1],
                            start=(k == 0), stop=(k == kt - 1))

                    o1 = wk.tile([128, D1], f32, tag="o1")
                    o1v = o1[:].rearrange("p (h c) -> p h c", c=HID)
                    recb = rec[:].unsqueeze(2).to_broadcast([128, HEADS, HID])
                    nc.vector.tensor_tensor(
                        o1v, ps_c[:].rearrange("p (h c) -> p h c", c=HID),
                        recb, op=ALU.mult)
                    nc.vector.tensor_add(o1[:], o1[:], b1s[:])
                    # elu = max(x,0) + exp(min(x,0)) - 1, fp32, bf16 out
                    zb = zc[:].to_broadcast([128, D1])
                    tn = wk.tile([128, D1], f32, tag="tn")
                    nc.vector.tensor_tensor(tn[:], o1[:], zb, op=ALU.min)
                    nc.scalar.activation(tn[:], tn[:], AF.Exp)
                    o1b = wk.tile([128, D1], bf16, tag="o1b")
                    nc.vector.tensor_tensor(o1[:], o1[:], zb, op=ALU.max)
                    nc.vector.scalar_tensor_tensor(
                        o1b[:], tn[:], -1.0, o1[:], op0=ALU.add, op1=ALU.add)

                    # fused phase D: z2 rows for this tile
                    tts = []
                    for kk in range(DK):
                        ps_t = pstp.tile([128, 128], bf16, tag="tr")
                        nc.tensor.transpose(
                            ps_t[:], o1b[:, kk * 128:(kk + 1) * 128], ids[:])
                        tt = wk.tile([128, 128], bf16, tag=f"tt{kk}")
                        nc.vector.tensor_copy(tt[:], ps_t[:])
                        tts.append(tt)
                    ps_d = pstp.tile([128, TW2], f32, tag="mm2")
                    for kk in range(DK):
                        nc.tensor.matmul(
                            ps_d[:], lhsT=tts[kk][:],
                            rhs=w2s[:, kk * TW2:(kk + 1) * TW2],
                            start=(kk == 0), stop=(kk == DK - 1))
                    hd = wk.tile([128, G2], bf16, tag="hd")
                    nc.vector.tensor_copy(hd[:], ps_d[:, 0:G2])
                    if t == 0:
                        nc.vector.tensor_add(hd[:, NCLS:G2], hd[:, NCLS:G2],
                                             padms[:, 0:1])
                    if t == NT - 1:
                        nc.vector.tensor_add(hd[:, NCLS:G2], hd[:, NCLS:G2],
                                             padms[:, HEADS:HEADS + 1])
                    nc.sync.dma_start(
                        out=loc2[j][tj * 128:(tj + 1) * 128, 0:G2], in_=hd[:])
                    nc.vector.tensor_copy(ald2s[:, t:t + 1],
                                          ps_d[:, G2:G2 + 1])
                    if tj == TPC - 1:
                        with nc.named_scope("l2_ag"):
                            nc.gpsimd.collective_compute(
                                "AllGather", mybir.AluOpType.bypass,
                                replica_groups=rg,
                                ins=[loc2[j][:]], outs=[t2i[j][:]],
                            )
                        with nc.named_scope("l2_restride"):
                            nc.sync.dma_start(
                                out=tab2[j * CHB:(j + 1) * CHB, :],
                                in_=t2i[j][:])

            # ---- phase F: layer-2 edge pass ----
            with nc.named_scope("l2_edges"):
                for t in range(NT):
                    klo, khi, kt = KLO[t], KHI[t], K[t]
                    g = gvp.tile([128, kt * G2], bf16, tag="g2")
                    gv = g[:].rearrange("p (k c) -> p k c", c=G2)
                    _dma_gather_raw(nc, gv[:, 0:klo, :], tab2[0:HALF, 0:G2],
                                    ilos[:, LOFF[t] * 8:LOFF[t + 1] * 8],
                                    klo * 128, G2, S2,
                                    queue_num=(2 * t) % NSWQ)
                    _dma_gather_raw(nc, gv[:, klo:kt, :],
                                    tab2[HIBASE:NPALL, 0:G2],
                                    ihis[:, HOFF[t] * 8:HOFF[t + 1] * 8],
                                    khi * 128, G2, S2,
                                    queue_num=(2 * t + 1) % NSWQ)

                    e2 = wk.tile([128, kt], f32, tag="e2")
                    adb2 = ald2s[:, t:t + 1].to_broadcast([128, kt])
                    nc.vector.tensor_tensor(e2[:], gv[:, :, NCLS], adb2,
                                            op=ALU.add)
                    nc.vector.scalar_tensor_tensor(
                        e2[:], e2[:], 0.2, e2[:], op0=ALU.mult, op1=ALU.max)
                    # exp -> bf16 back into the as2 column slot
                    nc.scalar.activation(gv[:, :, NCLS], e2[:], AF.Exp)

                    exb2 = gv[:, :, NCLS].unsqueeze(2) \
                        .to_broadcast([128, kt, NCLS])
                    nc.vector.tensor_mul(gv[:, :, 0:NCLS], gv[:, :, 0:NCLS],
                                         exb2)
                    ps_f = psp.tile([128, G2], f32, tag="mm")
                    for k in range(kt):
                        nc.tensor.matmul(ps_f[:], lhsT=ids[:],
                                         rhs=gv[:, k, 0:G2],
                                         start=(k == 0), stop=(k == kt - 1))
                    den2 = wk.tile([128, 1], f32, tag="den2")
                    nc.vector.tensor_copy(den2[:], ps_f[:, NCLS:G2])
                    rec2 = wk.tile([128, 1], f32, tag="rec2")
                    nc.vector.reciprocal(rec2[:], den2[:])

                    o2 = wk.tile([128, NCLS], f32, tag="o2")
                    nc.vector.tensor_tensor(o2[:], ps_f[:, 0:NCLS],
                                            rec2[:].to_broadcast([128, NCLS]),
                                            op=ALU.mult)
                    nc.vector.tensor_add(o2[:], o2[:], b2s[:])

                    nrmax = wk.tile([128, 1], f32, tag="nrmax")
                    nc.vector.tensor_reduce(nrmax[:], o2[:], axis=AX.X,
                                            op=ALU.max, negate=True)
                    sh = shs[:, t * NCLS:(t + 1) * NCLS]
                    nc.vector.tensor_tensor(sh, o2[:],
                                            nrmax[:].to_broadcast([128, NCLS]),
                                            op=ALU.add)
                    exs = wk.tile([128, NCLS], f32, tag="exs")
                    nc.scalar.activation(exs[:], sh, AF.Exp,
                                         accum_out=ssums[:, t:t + 1])

            # ---- phase G: batched log + final output ----
            with nc.named_scope("logsoftmax"):
                nc.scalar.activation(lgss[:], ssums[:], AF.Ln)
                for t in range(NT):
                    outf = wk.tile([128, NCLS], f32, tag="outf")
                    nc.vector.tensor_tensor(
                        outf[:], shs[:, t * NCLS:(t + 1) * NCLS],
                        lgss[:, t:t + 1].to_broadcast([128, NCLS]),
                        op=ALU.subtract)
                    nc.sync.dma_start(out=outp[t * 128:(t + 1) * 128, :],
                                      in_=outf[:])

    nc.compile()
    return nc


def _get_program(cfg):
    key = (cfg["KLO"], cfg["KHI"])
    if key not in _BUILD_CACHE:
        _BUILD_CACHE[key] = _build_program(cfg["KLO"], cfg["KHI"])
    return _BUILD_CACHE[key]


def kernel(**inputs):
    cfg, in_maps, node_at = _host_prep(
        inputs["x"], inputs["edge_index"], inputs["W1"], inputs["a_src1"],
        inputs["a_dst1"], inputs["b1"], inputs["W2"], inputs["a_src2"],
        inputs["a_dst2"], inputs["b2"],
    )
    nc = _get_program(cfg)

    from concourse.bass_utils import run_bass_kernel_spmd

    trace = bool(int(os.environ.get("GAT_PROFILE", "0")))
    if trace:
        trace = _register_trace_hook()
    res = run_bass_kernel_spmd(nc, in_maps, list(range(C)), trace=trace)
    if trace and res.exec_time_ns is not None:
        print(f"HW exec time: {res.exec_time_ns} ns", flush=True)

    out = np.empty((N, NCLS), np.float32)
    for c in range(C):
        r = np.asarray(res.results[c]["outp"], np.float32)
        m = node_at[c] >= 0
        out[node_at[c][m]] = r[m]
    return out
